# revision 30
# baseline (speedup 1.0000x reference)
"""Sliding-window causal self-attention (B=2, T=2048, C=1024, H=16, Dh=64,
window=256) + QKV/out projections, sharded over 8 NeuronCores as
data-parallel over B (2) x tensor-parallel over head groups (4 heads/core).

v2 layout: scores are computed TRANSPOSED (S^T = K_rot^T-contract vs Q_rot:
keys on partitions, queries on the free axis) so softmax probabilities come
out of exp() already in the [k, q] layout that PV consumes -- no P-transpose
matmuls and no PSUM->SBUF staging copy of P^T.  PV is computed token-major
(out = pT^T @ v_aug) with a ones-column appended to v, which makes the
softmax denominator land as a per-partition column of the PV accumulator;
normalization then rides the PSUM->SBUF eviction as an activation-scale copy.
A single PE transpose per (head-pair, query-tile) puts the attention output
back in feature-major layout for the output projection.

Input weight/activation DMAs are chunked per 128-row contraction slice and
the QKV projection runs contraction-outer over 8 interleaved PSUM
accumulators, so the tensor engine starts as soon as the first chunk lands
instead of waiting for the full 7 MB load.
"""

import math

import numpy as np

B = 2
T = 2048
C = 1024
H = 16
DH = 64
WINDOW = 256
HEADS_PER_CORE = 4
N_CORES = 8
QT = T // 128  # 16 tiles of 128 tokens
FQ = HEADS_PER_CORE * DH  # 256 local features
VW = 65  # v columns per head incl. ones column for the denominator

_PROGRAM = None  # compile once per process


def _emit(nc, tc, aps, ctx):
    from contextlib import ExitStack

    from concourse import mybir

    f32 = mybir.dt.float32
    bf16 = mybir.dt.bfloat16
    Exp = mybir.ActivationFunctionType.Exp
    is_gt = mybir.AluOpType.is_gt
    is_ge = mybir.AluOpType.is_ge

    xT, wT, woT, cos4, sin4, ident, y = (
        aps["xT"], aps["wT"], aps["woT"], aps["cos4"], aps["sin4"],
        aps["ident"], aps["y"],
    )

    consts = ctx.enter_context(tc.tile_pool(name="consts", bufs=1))
    stage = ctx.enter_context(tc.tile_pool(name="stage", bufs=1))
    tmp = ctx.enter_context(tc.tile_pool(name="tmp", bufs=4))

    # ---- chunked resident inputs: contraction slice kc = rows [128kc,128kc+128)
    xc = [consts.tile([128, T], bf16, tag=f"xc{kc}", name=f"xc{kc}")
          for kc in range(8)]
    wc = [consts.tile([128, 768], bf16, tag=f"wc{kc}", name=f"wc{kc}")
          for kc in range(8)]
    for kc in range(8):
        nc.sync.dma_start(out=xc[kc], in_=xT[kc * 128:(kc + 1) * 128, :])
        nc.sync.dma_start(out=wc[kc], in_=wT[kc * 128:(kc + 1) * 128, :])
    cos_sb = consts.tile([128, T], bf16, tag="cos")
    nc.sync.dma_start(out=cos_sb, in_=cos4)
    sin_sb = consts.tile([128, T], bf16, tag="sin")
    nc.sync.dma_start(out=sin_sb, in_=sin4)
    id_sb = consts.tile([128, 128], bf16, tag="ident")
    nc.sync.dma_start(out=id_sb, in_=ident)
    woT_sb = consts.tile([128, 2 * C], bf16, tag="woT")
    nc.sync.dma_start(
        out=woT_sb.rearrange("p (kc e) -> p kc e", kc=2),
        in_=woT.rearrange("(kc p) e -> p kc e", p=128),
    )

    # ---- persistent intermediates ----
    # pre-RoPE blocks [q_x1, q_x2, k_x1, k_x2], each [128=(4h x 32d), T]
    pre = [stage.tile([128, T], bf16, tag=f"pre{i}", name=f"pre{i}")
           for i in range(4)]
    rot = [stage.tile([128, T], bf16, tag=f"rot{i}", name=f"rot{i}")
           for i in range(4)]
    qhT = stage.tile([64, HEADS_PER_CORE * T], bf16, tag="qhT")
    khT = stage.tile([64, HEADS_PER_CORE * T], bf16, tag="khT")
    # v_aug: per key tile kt, per head: 64 v features + a ones column
    v_sb = stage.tile([128, QT * 4 * VW], bf16, tag="v")
    attnT = stage.tile([128, 2 * T], bf16, tag="attnT")

    # ones columns of v_aug (written once, before the v copies)
    nc.vector.memset(
        v_sb.rearrange("p (g c) -> p g c", c=VW)[:, :, 64:65], 1.0)

    # ---- phase 1: q,k projection, contraction-outer over 8 accumulators ----
    def rope(split):
        tsl = slice(split * 512, (split + 1) * 512)
        for pair in range(2):  # 0 -> q, 1 -> k
            x1, x2 = pre[2 * pair][:, tsl], pre[2 * pair + 1][:, tsl]
            r1, r2 = rot[2 * pair][:, tsl], rot[2 * pair + 1][:, tsl]
            t1 = tmp.tile([128, 512], bf16, tag="t1")
            t2 = tmp.tile([128, 512], bf16, tag="t2")
            t3 = tmp.tile([128, 512], bf16, tag="t3")
            t4 = tmp.tile([128, 512], bf16, tag="t4")
            nc.vector.tensor_mul(t1, x1, cos_sb[:, tsl])
            nc.vector.tensor_mul(t2, x2, sin_sb[:, tsl])
            nc.vector.tensor_sub(r1, t1, t2)
            nc.vector.tensor_mul(t3, x2, cos_sb[:, tsl])
            nc.vector.tensor_mul(t4, x1, sin_sb[:, tsl])
            nc.vector.tensor_add(r2, t3, t4)

    def repack(thalf):
        # head-contiguous [64, (h t)] via SBUF->SBUF DMA, one 1024-token half
        # at a time so the first half unblocks attention while the second
        # projection half is still in flight; issued on the scalar queue to
        # keep descriptor generation off the sync queue's critical chain
        tsl = slice(thalf * 1024, (thalf + 1) * 1024)
        for hl in range(HEADS_PER_CORE):
            csl = slice(hl * T + thalf * 1024, hl * T + (thalf + 1) * 1024)
            for half in range(2):
                nc.sync.dma_start(
                    out=qhT[half * 32:(half + 1) * 32, csl],
                    in_=rot[half][hl * 32:(hl + 1) * 32, tsl],
                )
                nc.sync.dma_start(
                    out=khT[half * 32:(half + 1) * 32, csl],
                    in_=rot[2 + half][hl * 32:(hl + 1) * 32, tsl],
                )

    def v_proj(kt, pool=None, tag="ot"):
        vacc = (pool or pov).tile([128, 4 * VW], f32, tag=tag, name=f"vacc{kt}")
        for kc in range(8):
            nc.tensor.matmul(
                vacc[:, :FQ],
                lhsT=xc[kc][:, kt * 128:(kt + 1) * 128],
                rhs=wc[kc][:, 512:768],
                start=(kc == 0),
                stop=(kc == 7),
            )
        nc.scalar.copy(
            v_sb[:, kt * 4 * VW:(kt + 1) * 4 * VW]
            .rearrange("p (h c) -> p h c", h=4)[:, :, 0:64],
            vacc[:, :FQ].rearrange("p (h d) -> p h d", h=4),
        )

    with ExitStack() as phase1:
        pmm = phase1.enter_context(tc.tile_pool(name="pmm", bufs=8, space="PSUM"))
        for half in range(2):
            accs = [pmm.tile([128, 512], f32, tag="mm", name=f"acc{half}_{j}")
                    for j in range(8)]
            for kc in range(8):  # contraction-outer: chunk kc feeds all 8 chains
                for j in range(8):
                    split, blk = 2 * half + j // 4, j % 4
                    nc.tensor.matmul(
                        accs[j],
                        lhsT=wc[kc][:, blk * 128:(blk + 1) * 128],
                        rhs=xc[kc][:, split * 512:(split + 1) * 512],
                        start=(kc == 0),
                        stop=(kc == 7),
                    )
            for j in range(8):
                split, blk = 2 * half + j // 4, j % 4
                nc.scalar.copy(pre[blk][:, split * 512:(split + 1) * 512], accs[j])
            if half == 1:
                # first v tiles ride the draining projection accumulators so
                # the tensor engine has work while rope/repack run
                for kt in range(6):
                    v_proj(kt, pool=pmm, tag="mm")
            rope(2 * half)
            rope(2 * half + 1)
            repack(half)

    # ---- phase 2: v tiles + banded attention (S^T layout) + out-proj ----
    # Software-pipelined: iteration kt runs QK/exp/mask for key tile kt and
    # the PV/normalize/transpose/out-proj block for qt = kt-1, so the tensor
    # engine never waits on the scalar/gpsimd softmax of the current tile.
    big = ctx.enter_context(tc.tile_pool(name="big", bufs=4, space="PSUM"))
    pov = ctx.enter_context(tc.tile_pool(name="pov", bufs=2, space="PSUM"))
    ptr = ctx.enter_context(tc.tile_pool(name="ptr", bufs=2, space="PSUM"))
    ptp = ctx.enter_context(tc.tile_pool(name="ptp", bufs=28))
    small = ctx.enter_context(tc.tile_pool(name="small", bufs=4))
    otnp = ctx.enter_context(tc.tile_pool(name="otnp", bufs=4))
    ysbp = ctx.enter_context(tc.tile_pool(name="ysbp", bufs=3))

    pts = {}
    ysb_pair = [None]

    def score_tile(kt):
        # scores S^T[k, q] for key tile kt against its in-window query tiles
        w = min(QT - kt, 3) * 128
        for hl in range(HEADS_PER_CORE):
            s = big.tile([128, 512], f32, tag="big", name=f"s{kt}_{hl}")
            nc.tensor.matmul(
                s[:, :w],
                lhsT=khT[:, hl * T + kt * 128:hl * T + (kt + 1) * 128],
                rhs=qhT[:, hl * T + kt * 128:hl * T + kt * 128 + w],
                start=True,
                stop=True,
            )
            pt = ptp.tile([128, 384], bf16, tag="pt", name=f"pt{kt}_{hl}")
            pts[(hl, kt)] = pt
            nc.scalar.activation(pt[:, :w], s[:, :w], Exp)
            # banded mask, multiplicative on P^T: diagonal block keeps q >= k,
            # the +2 off-diagonal block keeps k > q; middle block is all-valid
            nc.gpsimd.affine_select(
                pt[:, 0:128], pt[:, 0:128],
                pattern=[[1, 128]], compare_op=is_ge, fill=0.0,
                base=0, channel_multiplier=-1,
            )
            if kt + 2 < QT:
                nc.gpsimd.affine_select(
                    pt[:, 256:384], pt[:, 256:384],
                    pattern=[[-1, 128]], compare_op=is_gt, fill=0.0,
                    base=0, channel_multiplier=1,
                )

    def out_tile(qt):
        # PV for query tile qt (token-major, denominator in column 64)
        kt0 = max(qt - 2, 0)
        ot = pov.tile([128, 4 * VW], f32, tag="ot", name=f"ot{qt}")
        for hl in range(HEADS_PER_CORE):
            for a, k2 in enumerate(range(kt0, qt + 1)):
                nc.tensor.matmul(
                    ot[:, hl * VW:(hl + 1) * VW],
                    lhsT=pts[(hl, k2)][:, (qt - k2) * 128:(qt - k2 + 1) * 128],
                    rhs=v_sb[:, k2 * 4 * VW + hl * VW:k2 * 4 * VW + (hl + 1) * VW],
                    start=(a == 0),
                    stop=(a == qt - kt0),
                )
        rc = small.tile([128, 4], f32, tag="rc")
        nc.vector.reciprocal(
            rc, ot.rearrange("p (h c) -> p h c", h=4)[:, :, 64:65])

        # normalize + evict + transpose back to feature-major attnT
        if qt % 2 == 0:
            ysb_pair[0] = ysbp.tile([128, 2 * C], bf16, tag="ysb",
                                    name=f"ysb{qt}")
        ysb = ysb_pair[0][:, (qt % 2) * C:(qt % 2 + 1) * C]
        otns = []
        for pair in range(2):
            otn = otnp.tile([128, 128], bf16, tag="otn", name=f"otn{qt}_{pair}")
            otns.append(otn)
            for h2 in range(2):
                hl = 2 * pair + h2
                eng = nc.scalar.mul if h2 == 0 else (
                    lambda o, i, m: nc.vector.tensor_scalar_mul(o, i, m))
                eng(
                    otn[:, h2 * 64:(h2 + 1) * 64],
                    ot[:, hl * VW:hl * VW + 64],
                    rc[:, hl:hl + 1],
                )
        for pair in range(2):
            tr = ptr.tile([128, 128], bf16, tag="tr")
            nc.tensor.transpose(tr, otns[pair], id_sb)
            nc.vector.tensor_copy(
                attnT[:, pair * T + qt * 128:pair * T + (qt + 1) * 128], tr)
        for nh in range(2):
            acc = big.tile([128, 512], f32, tag="big", name=f"op{qt}_{nh}")
            for kc in range(2):
                nc.tensor.matmul(
                    acc,
                    lhsT=attnT[:, kc * T + qt * 128:kc * T + (qt + 1) * 128],
                    rhs=woT_sb[:, kc * C + nh * 512:kc * C + (nh + 1) * 512],
                    start=(kc == 0),
                    stop=(kc == 1),
                )
            if nh == 0:
                nc.scalar.copy(ysb[:, nh * 512:(nh + 1) * 512], acc)
            else:
                nc.vector.tensor_copy(ysb[:, nh * 512:(nh + 1) * 512], acc)
        if qt % 2 == 1:
            nc.sync.dma_start(
                out=y[(qt - 1) * 128:(qt + 1) * 128, :]
                .rearrange("(g p) e -> p g e", p=128),
                in_=ysb_pair[0].rearrange("p (g e) -> p g e", g=2),
            )

    for kt in range(QT):
        score_tile(kt)
        if kt + 6 < QT:
            v_proj(kt + 6)
        if kt >= 3:
            out_tile(kt - 3)
    out_tile(QT - 3)
    out_tile(QT - 2)
    out_tile(QT - 1)


def _build_program():
    import concourse.tile as tile
    from concourse import bacc, mybir

    bf16 = mybir.dt.bfloat16

    nc = bacc.Bacc("TRN2", target_bir_lowering=False, debug=False,
                   num_devices=N_CORES)
    aps = {
        "xT": nc.dram_tensor("xT", [C, T], bf16, kind="ExternalInput").ap(),
        "wT": nc.dram_tensor("wT", [C, 768], bf16, kind="ExternalInput").ap(),
        "woT": nc.dram_tensor("woT", [FQ, C], bf16, kind="ExternalInput").ap(),
        "cos4": nc.dram_tensor("cos4", [128, T], bf16, kind="ExternalInput").ap(),
        "sin4": nc.dram_tensor("sin4", [128, T], bf16, kind="ExternalInput").ap(),
        "ident": nc.dram_tensor("ident", [128, 128], bf16, kind="ExternalInput").ap(),
        "y": nc.dram_tensor("y", [T, C], bf16, kind="ExternalOutput").ap(),
    }
    from contextlib import ExitStack

    with tile.TileContext(nc) as tc, ExitStack() as ctx:
        _emit(nc, tc, aps, ctx)
    nc.compile()
    return nc


def _get_program():
    global _PROGRAM
    if _PROGRAM is None:
        _PROGRAM = _build_program()
    return _PROGRAM


def _host_inputs(x, w_qkv, w_out):
    import ml_dtypes

    bf16 = ml_dtypes.bfloat16
    x = np.asarray(x, np.float32)
    w_qkv = np.asarray(w_qkv, np.float32)
    w_out = np.asarray(w_out, np.float32)

    wq, wk, wv = w_qkv[0:C], w_qkv[C:2 * C], w_qkv[2 * C:3 * C]
    scale = 1.0 / math.sqrt(DH)

    # RoPE tables (transposed, tiled over the 4 heads of a block)
    inv_freq = 1.0 / (10000.0 ** (np.arange(0, DH, 2, dtype=np.float32) / DH))
    freqs = np.outer(np.arange(T, dtype=np.float32), inv_freq)  # [T, 32]
    cos4 = np.ascontiguousarray(np.tile(np.cos(freqs).T, (4, 1))).astype(bf16)
    sin4 = np.ascontiguousarray(np.tile(np.sin(freqs).T, (4, 1))).astype(bf16)
    ident = np.eye(128, dtype=np.float32).astype(bf16)

    xT = [np.ascontiguousarray(x[b].T).astype(bf16) for b in range(B)]

    in_maps = []
    for core in range(N_CORES):
        b, g = divmod(core, 4)
        hs = range(4 * g, 4 * g + 4)
        rows = []
        for half in range(2):  # q_x1, q_x2
            rows.append(np.concatenate(
                [wq[h * DH + 32 * half:h * DH + 32 * half + 32] for h in hs]) * scale)
        for half in range(2):  # k_x1, k_x2
            rows.append(np.concatenate(
                [wk[h * DH + 32 * half:h * DH + 32 * half + 32] for h in hs]))
        rows.append(wv[g * FQ:(g + 1) * FQ])
        wmat = np.concatenate(rows)  # [768, C]
        wT = np.ascontiguousarray(wmat.T).astype(bf16)
        woT = np.ascontiguousarray(w_out[:, g * FQ:(g + 1) * FQ].T).astype(bf16)
        in_maps.append({
            "xT": xT[b], "wT": wT, "woT": woT,
            "cos4": cos4, "sin4": sin4, "ident": ident,
        })
    return in_maps


def kernel(x, w_qkv, w_out, _trace=False):
    from concourse import bass_utils

    nc = _get_program()
    in_maps = _host_inputs(x, w_qkv, w_out)
    res = bass_utils.run_bass_kernel_spmd(
        nc, in_maps, core_ids=list(range(N_CORES)), trace=_trace,
    )
    parts = [res.results[core]["y"].astype(np.float32) for core in range(N_CORES)]
    out = np.stack([
        parts[0] + parts[1] + parts[2] + parts[3],
        parts[4] + parts[5] + parts[6] + parts[7],
    ])
    if _trace:
        return out, res
    return out
